# revision 6
# baseline (speedup 1.0000x reference)
"""BERT self-attention (B=2, S=2048, D=768, H=12, DH=64) on 8 trn2 NeuronCores.

Sharding: data parallel on batch x tensor parallel on heads. Core c handles
batch b = c // 4 and heads h0..h0+2 with h0 = 3 * (c % 4) — 24 (b, h) units,
3 per core.

Per-core kernel (all layouts chosen so nothing is transposed on-chip):
  - hidden^T [768, 2048] arrives k-major; W^T slices arrive as stationary
    groups. Q^T/K^T [64, 2048] come straight out of the projection matmuls
    (head dim on partitions); V comes out token-major [2048, 64] by swapping
    stationary/moving operands. Biases are folded in as one extra K=1
    accumulating matmul per output tile (bias x ones-row rank-1 update).
  - Scores are computed transposed: S^T[j, i] = K^T.T @ Q^T per 128-key block,
    so the softmax probs are already key-major for the P @ V contraction.
  - exp runs on ScalarE straight out of PSUM with the 1/sqrt(DH) scale and the
    additive attention mask fused into the activation's scale/bias. No max
    subtraction: scores here are ~N(0, 1) + mask, far from fp32 exp overflow.
  - The softmax denominator is row 64 of the P @ V matmul: V is augmented with
    a ones column, so sum_j exp(S^T[j, i]) accumulates alongside the context.
  - Normalize: reciprocal of the denominator row, broadcast across the 64 head
    dims with a K=1 matmul, multiply on VectorE while draining PSUM.
Output per core is head-major transposed [3, 64, 2048]; the host assembles the
full [B, S, D] tensor (pure unsharding/layout, no arithmetic).
"""

import numpy as np

import concourse.bass as bass
import concourse.mybir as mybir
import concourse.tile as tile
from concourse import bacc
from concourse.bass import ts, ds
from concourse.bass_utils import run_bass_kernel_spmd

B, S, D = 2, 2048, 768
H, DH = 12, 64
NH = 3            # heads per core
N_CORES = 8
KC = D // 128     # contraction chunks (6)
NJ = S // 128     # key blocks (16)
IB = 1024         # query block (i) processed per exp/PV round
PROB_DT = mybir.dt.bfloat16   # dtype of softmax numerators (exp output)
TRACE = False     # set True (from test.py) to capture an NTFF profile
LAST_RESULT = {}  # exec_time_ns etc. for test.py

f32 = mybir.dt.float32
AF = mybir.ActivationFunctionType

_NC_CACHE = None


def build_nc():
    nc = bacc.Bacc("TRN2", target_bir_lowering=False, debug=False,
                   num_devices=N_CORES)
    hidT_d = nc.dram_tensor("hidT", [128, KC, S], f32, kind="ExternalInput")
    wT_d = nc.dram_tensor("wT", [128, KC, 576], f32, kind="ExternalInput")
    bias_d = nc.dram_tensor("biasrow", [1, 576], f32, kind="ExternalInput")
    mask_d = nc.dram_tensor("maskT", [128, NJ], f32, kind="ExternalInput")
    out_d = nc.dram_tensor("out", [NH, DH, S], f32, kind="ExternalOutput")

    with tile.TileContext(nc) as tc:
        with (
            tc.tile_pool(name="const", bufs=1) as cpool,
            tc.tile_pool(name="proj", bufs=1) as proj,
        ):
            ones = cpool.tile([1, 512], f32)
            nc.vector.memset(ones[:], 1.0)
            biasrow = cpool.tile([1, 576], f32)
            nc.sync.dma_start(biasrow[:], bias_d[:])
            maskT = cpool.tile([128, NJ], f32)
            nc.sync.dma_start(maskT[:], mask_d[:])

            # qk2 rows 0:64 = Q^T (drained), rows 64:128 = copy of Q^T; k2
            # rows 64:128 = K^T. The score matmul runs on partitions 64:128
            # (engines can't move data across partitions; psum rows 64:128
            # land there and a SB->SB DMA duplicates Q^T up).
            qk2 = proj.tile([128, NH, S], f32)
            k2 = proj.tile([128, NH, S], f32)
            vAug = proj.tile([128, NH, NJ, DH + 1], PROB_DT)
            nc.vector.memset(vAug[:], 1.0)  # col DH stays 1.0 -> denominators

            # ---- QKV projection phase ----
            with (
                tc.tile_pool(name="hid", bufs=1) as hpool,
                tc.tile_pool(name="wts", bufs=1) as wpool,
                tc.tile_pool(name="psQK", bufs=3, space="PSUM") as psQK,
                tc.tile_pool(name="psV", bufs=3, space="PSUM") as psV,
            ):
                hidT = hpool.tile([128, KC, S], f32)
                nc.sync.dma_start(hidT[:], hidT_d[:])
                wT = wpool.tile([128, KC, 576], f32)
                nc.sync.dma_start(wT[:], wT_d[:])

                # Q^T/K^T: stationary = [Wq_h^T | Wk_h^T] (128 cols), moving
                # = hidden^T -> psum rows 0:64 = Q^T, 64:128 = K^T.
                for h in range(NH):
                    for t in range(S // 512):
                        ps = psQK.tile([128, 512], f32)
                        for c in range(KC):
                            nc.tensor.matmul(
                                ps[:], wT[:, c, ts(h, 128)],
                                hidT[:, c, ts(t, 512)],
                                start=(c == 0), stop=False)
                        nc.tensor.matmul(  # + [bq|bk] x ones  (K=1)
                            ps[:], biasrow[0:1, ts(h, 128)], ones[0:1, 0:512],
                            start=False, stop=True)
                        nc.vector.tensor_copy(qk2[0:64, h, ts(t, 512)],
                                              ps[0:64, :])
                        nc.vector.tensor_copy(k2[64:128, h, ts(t, 512)],
                                              ps[64:128, :])
                        nc.sync.dma_start(qk2[64:128, h, ts(t, 512)],
                                          qk2[0:64, h, ts(t, 512)])

                # V token-major: stationary = hidden^T chunk (tokens as cols),
                # moving = Wv^T all 3 heads (192 cols).
                for t in range(NJ):
                    ps = psV.tile([128, 192], f32)
                    for c in range(KC):
                        nc.tensor.matmul(
                            ps[:], hidT[:, c, ts(t, 128)], wT[:, c, 384:576],
                            start=(c == 0), stop=False)
                    nc.tensor.matmul(  # + ones x bv  (K=1)
                        ps[:], ones[0:1, 0:128], biasrow[0:1, 384:576],
                        start=False, stop=True)
                    nc.vector.tensor_copy(
                        vAug[:, :, t, 0:DH],
                        ps[:].rearrange("p (h d) -> p h d", h=NH))

            # ---- attention phase ----
            with (
                tc.tile_pool(name="expS", bufs=2) as epool,
                tc.tile_pool(name="psS", bufs=2, space="PSUM") as psS,
                tc.tile_pool(name="psC", bufs=2, space="PSUM") as psC,
                tc.tile_pool(name="psB", bufs=2, space="PSUM") as psB,
                tc.tile_pool(name="den", bufs=4) as dpool,
                tc.tile_pool(name="rb", bufs=3) as rpool,
                tc.tile_pool(name="ost", bufs=3) as opool,
            ):
                for h in range(NH):
                    for ib in range(S // IB):
                        eS = epool.tile([128, NJ, IB], PROB_DT, tag="eS")
                        for j in range(NJ):
                            ps = psS.tile([128, IB], f32, tag="psS")
                            for n in range(IB // 512):
                                nc.tensor.matmul(
                                    ps[:, ts(n, 512)],
                                    k2[64:128, h, ts(j, 128)],
                                    qk2[64:128, h, ds(ib * IB + n * 512, 512)],
                                    start=True, stop=True)
                            # exp(S/8 + mask_j), mask as per-partition bias
                            nc.scalar.activation(
                                eS[:, j, :], ps[:], AF.Exp,
                                bias=maskT[:, j:j + 1], scale=0.125)
                        for it in range(IB // 512):
                            pc = psC.tile([DH + 1, 512], f32, tag="psC")
                            for j in range(NJ):
                                nc.tensor.matmul(
                                    pc[:], vAug[:, h, j, :],
                                    eS[:, j, ts(it, 512)],
                                    start=(j == 0), stop=(j == NJ - 1))
                            dr = dpool.tile([1, 512], f32, tag="den")
                            nc.vector.reciprocal(dr[:], pc[DH:DH + 1, :])
                            pb = psB.tile([64, 512], f32, tag="psB")
                            nc.tensor.matmul(pb[:], ones[0:1, 0:DH], dr[:],
                                             start=True, stop=True)
                            rB = rpool.tile([64, 512], f32, tag="rb")
                            nc.vector.tensor_copy(rB[:], pb[:])
                            o = opool.tile([64, 512], f32, tag="ost")
                            nc.vector.tensor_mul(o[:], pc[0:DH, :], rB[:])
                            nc.sync.dma_start(
                                out_d[h, :, ds(ib * IB + it * 512, 512)], o[:])
    nc.compile()
    return nc


def _prep_core_inputs(c, hidden_states, attention_mask, Wq, bq, Wk, bk, Wv, bv):
    b, h0 = c // 4, NH * (c % 4)
    rows = slice(h0 * DH, (h0 + NH) * DH)
    Wq_s, Wk_s, Wv_s = Wq[rows], Wk[rows], Wv[rows]      # [192, 768] each
    groups = []
    for h in range(NH):
        groups.append(Wq_s[h * DH:(h + 1) * DH])
        groups.append(Wk_s[h * DH:(h + 1) * DH])
    groups.append(Wv_s)
    big = np.concatenate(groups, axis=0)                 # [576, 768]
    wT = np.ascontiguousarray(
        big.T.reshape(KC, 128, 576).transpose(1, 0, 2))  # [128, KC, 576]
    hidT = np.ascontiguousarray(
        hidden_states[b].T.reshape(KC, 128, S).transpose(1, 0, 2))
    bias_groups = []
    for h in range(NH):
        bias_groups.append(bq[rows][h * DH:(h + 1) * DH])
        bias_groups.append(bk[rows][h * DH:(h + 1) * DH])
    bias_groups.append(bv[rows])
    biasrow = np.concatenate(bias_groups)[None, :]       # [1, 576]
    maskT = np.ascontiguousarray(
        attention_mask[b, 0, 0].reshape(NJ, 128).T)      # [128, NJ]
    return {"hidT": hidT, "wT": wT, "biasrow": biasrow.astype(np.float32),
            "maskT": maskT}


def kernel(hidden_states, attention_mask, Wq, bq, Wk, bk, Wv, bv):
    global _NC_CACHE, LAST_RESULT
    hidden_states = np.asarray(hidden_states, dtype=np.float32)
    attention_mask = np.asarray(attention_mask, dtype=np.float32)
    if _NC_CACHE is None:
        _NC_CACHE = build_nc()
    nc = _NC_CACHE
    in_maps = [
        _prep_core_inputs(c, hidden_states, attention_mask,
                          np.asarray(Wq), np.asarray(bq), np.asarray(Wk),
                          np.asarray(bk), np.asarray(Wv), np.asarray(bv))
        for c in range(N_CORES)
    ]
    res = run_bass_kernel_spmd(nc, in_maps, core_ids=list(range(N_CORES)),
                               trace=TRACE)
    LAST_RESULT = {"exec_time_ns": res.exec_time_ns,
                   "trace": res.instructions_and_trace}
    out = np.empty((B, S, H * DH), dtype=np.float32)
    for c in range(N_CORES):
        b, h0 = c // 4, NH * (c % 4)
        r = res.results[c]["out"]                        # [NH, DH, S]
        out[b, :, h0 * DH:(h0 + NH) * DH] = r.reshape(NH * DH, S).T
    return out


# revision 7
# speedup vs baseline: 2.0316x; 2.0316x over previous
"""BERT self-attention (B=2, S=2048, D=768, H=12, DH=64) on 8 trn2 NeuronCores.

Sharding: data parallel on batch x tensor parallel on heads. Core c handles
batch b = c // 4 and heads h0..h0+2 with h0 = 3 * (c % 4) — 24 (b, h) units,
3 per core.

Per-core kernel (all layouts chosen so nothing is transposed on-chip):
  - hidden^T [768, 2048] arrives k-major; W^T slices arrive as stationary
    groups. Q^T/K^T [64, 2048] come straight out of the projection matmuls
    (head dim on partitions); V comes out token-major [2048, 64] by swapping
    stationary/moving operands. Biases are folded in as one extra K=1
    accumulating matmul per output tile (bias x ones-row rank-1 update).
  - Scores are computed transposed: S^T[j, i] = K^T.T @ Q^T per 128-key block,
    so the softmax probs are already key-major for the P @ V contraction.
  - exp runs on ScalarE straight out of PSUM with the 1/sqrt(DH) scale and the
    additive attention mask fused into the activation's scale/bias. No max
    subtraction: scores here are ~N(0, 1) + mask, far from fp32 exp overflow.
  - The softmax denominator is row 64 of the P @ V matmul: V is augmented with
    a ones column, so sum_j exp(S^T[j, i]) accumulates alongside the context.
  - Normalize: reciprocal of the denominator row, broadcast across the 64 head
    dims with a K=1 matmul, multiply on VectorE while draining PSUM.
Output per core is head-major transposed [3, 64, 2048]; the host assembles the
full [B, S, D] tensor (pure unsharding/layout, no arithmetic).
"""

import numpy as np

import concourse.bass as bass
import concourse.mybir as mybir
import concourse.tile as tile
from concourse import bacc
from concourse.bass import ts, ds
from concourse.bass_utils import run_bass_kernel_spmd

B, S, D = 2, 2048, 768
H, DH = 12, 64
NH = 3            # heads per core
N_CORES = 8
KC = D // 128     # contraction chunks (6)
NJ = S // 128     # key blocks (16)
IB = 1024         # query block (i) processed per exp/PV round
MM_DT = mybir.dt.float16      # matmul operand dtype (psum accum stays f32)
TRACE = False     # set True (from test.py) to capture an NTFF profile
LAST_RESULT = {}  # exec_time_ns etc. for test.py

f32 = mybir.dt.float32
f16 = mybir.dt.float16
AF = mybir.ActivationFunctionType

_NC_CACHE = None


def build_nc():
    nc = bacc.Bacc("TRN2", target_bir_lowering=False, debug=False,
                   num_devices=N_CORES)
    hidT_d = nc.dram_tensor("hidT", [128, KC, S], MM_DT, kind="ExternalInput")
    wT_d = nc.dram_tensor("wT", [128, KC, 576], MM_DT, kind="ExternalInput")
    bias_d = nc.dram_tensor("biasrow", [1, 576], MM_DT, kind="ExternalInput")
    mask_d = nc.dram_tensor("maskT", [128, NJ], f32, kind="ExternalInput")
    out_d = nc.dram_tensor("out", [NH, DH, S], f32, kind="ExternalOutput")

    with tile.TileContext(nc) as tc:
        with (
            tc.tile_pool(name="const", bufs=1) as cpool,
            tc.tile_pool(name="proj", bufs=1) as proj,
        ):
            ones = cpool.tile([1, 512], MM_DT)
            nc.vector.memset(ones[:], 1.0)
            ones_f = cpool.tile([1, 64], f32)
            nc.vector.memset(ones_f[:], 1.0)
            biasrow = cpool.tile([1, 576], MM_DT)
            nc.sync.dma_start(biasrow[:], bias_d[:])
            maskT = cpool.tile([128, NJ], f32)
            nc.sync.dma_start(maskT[:], mask_d[:])

            # qk2 rows 0:64 = Q^T (drained), rows 64:128 = copy of Q^T; k2
            # rows 64:128 = K^T. The score matmul runs on partitions 64:128
            # (engines can't move data across partitions; psum rows 64:128
            # land there and a SB->SB DMA duplicates Q^T up).
            qk2 = proj.tile([128, NH, S], MM_DT)
            k2 = proj.tile([128, NH, S], MM_DT)
            vAug = proj.tile([128, NH, NJ, DH + 1], MM_DT)
            nc.vector.memset(vAug[:], 1.0)  # col DH stays 1.0 -> denominators

            # ---- QKV projection phase ----
            with (
                tc.tile_pool(name="hid", bufs=1) as hpool,
                tc.tile_pool(name="wts", bufs=1) as wpool,
                tc.tile_pool(name="psQK", bufs=3, space="PSUM") as psQK,
                tc.tile_pool(name="psV", bufs=3, space="PSUM") as psV,
            ):
                hidT = hpool.tile([128, KC, S], MM_DT)
                nc.sync.dma_start(hidT[:], hidT_d[:])
                wT = wpool.tile([128, KC, 576], MM_DT)
                nc.sync.dma_start(wT[:], wT_d[:])

                # Q^T/K^T: stationary = [Wq_h^T | Wk_h^T] (128 cols), moving
                # = hidden^T -> psum rows 0:64 = Q^T, 64:128 = K^T.
                for h in range(NH):
                    for t in range(S // 512):
                        ps = psQK.tile([128, 512], f32)
                        for c in range(KC):
                            nc.tensor.matmul(
                                ps[:], wT[:, c, ts(h, 128)],
                                hidT[:, c, ts(t, 512)],
                                start=(c == 0), stop=False)
                        nc.tensor.matmul(  # + [bq|bk] x ones  (K=1)
                            ps[:], biasrow[0:1, ts(h, 128)], ones[0:1, 0:512],
                            start=False, stop=True)
                        nc.vector.tensor_copy(qk2[0:64, h, ts(t, 512)],
                                              ps[0:64, :])
                        nc.vector.tensor_copy(k2[64:128, h, ts(t, 512)],
                                              ps[64:128, :])
                        nc.sync.dma_start(qk2[64:128, h, ts(t, 512)],
                                          qk2[0:64, h, ts(t, 512)])

                # V token-major: stationary = hidden^T chunk (tokens as cols),
                # moving = Wv^T all 3 heads (192 cols).
                for t in range(NJ):
                    ps = psV.tile([128, 192], f32)
                    for c in range(KC):
                        nc.tensor.matmul(
                            ps[:], hidT[:, c, ts(t, 128)], wT[:, c, 384:576],
                            start=(c == 0), stop=False)
                    nc.tensor.matmul(  # + ones x bv  (K=1)
                        ps[:], ones[0:1, 0:128], biasrow[0:1, 384:576],
                        start=False, stop=True)
                    nc.vector.tensor_copy(
                        vAug[:, :, t, 0:DH],
                        ps[:].rearrange("p (h d) -> p h d", h=NH))

            # ---- attention phase ----
            with (
                tc.tile_pool(name="expS", bufs=2) as epool,
                tc.tile_pool(name="psS", bufs=2, space="PSUM") as psS,
                tc.tile_pool(name="psC", bufs=2, space="PSUM") as psC,
                tc.tile_pool(name="psB", bufs=2, space="PSUM") as psB,
                tc.tile_pool(name="den", bufs=4) as dpool,
                tc.tile_pool(name="rb", bufs=3) as rpool,
                tc.tile_pool(name="ost", bufs=3) as opool,
            ):
                for h in range(NH):
                    for ib in range(S // IB):
                        eS = epool.tile([128, NJ, IB], MM_DT, tag="eS")
                        for j in range(NJ):
                            ps = psS.tile([128, IB], f32, tag="psS")
                            for n in range(IB // 512):
                                nc.tensor.matmul(
                                    ps[:, ts(n, 512)],
                                    k2[64:128, h, ts(j, 128)],
                                    qk2[64:128, h, ds(ib * IB + n * 512, 512)],
                                    start=True, stop=True)
                            # exp(S/8 + mask_j), mask as per-partition bias
                            nc.scalar.activation(
                                eS[:, j, :], ps[:], AF.Exp,
                                bias=maskT[:, j:j + 1], scale=0.125)
                        for it in range(IB // 512):
                            pc = psC.tile([DH + 1, 512], f32, tag="psC")
                            for j in range(NJ):
                                nc.tensor.matmul(
                                    pc[:], vAug[:, h, j, :],
                                    eS[:, j, ts(it, 512)],
                                    start=(j == 0), stop=(j == NJ - 1))
                            dr = dpool.tile([1, 512], f32, tag="den")
                            nc.vector.reciprocal(dr[:], pc[DH:DH + 1, :])
                            pb = psB.tile([64, 512], f32, tag="psB")
                            nc.tensor.matmul(pb[:], ones_f[0:1, 0:DH], dr[:],
                                             start=True, stop=True)
                            rB = rpool.tile([64, 512], f32, tag="rb")
                            nc.vector.tensor_copy(rB[:], pb[:])
                            o = opool.tile([64, 512], f32, tag="ost")
                            nc.vector.tensor_mul(o[:], pc[0:DH, :], rB[:])
                            nc.sync.dma_start(
                                out_d[h, :, ds(ib * IB + it * 512, 512)], o[:])
    nc.compile()
    return nc


def _prep_core_inputs(c, hidden_states, attention_mask, Wq, bq, Wk, bk, Wv, bv):
    b, h0 = c // 4, NH * (c % 4)
    rows = slice(h0 * DH, (h0 + NH) * DH)
    Wq_s, Wk_s, Wv_s = Wq[rows], Wk[rows], Wv[rows]      # [192, 768] each
    groups = []
    for h in range(NH):
        groups.append(Wq_s[h * DH:(h + 1) * DH])
        groups.append(Wk_s[h * DH:(h + 1) * DH])
    groups.append(Wv_s)
    big = np.concatenate(groups, axis=0)                 # [576, 768]
    wT = np.ascontiguousarray(
        big.T.reshape(KC, 128, 576).transpose(1, 0, 2)).astype(np.float16)
    hidT = np.ascontiguousarray(
        hidden_states[b].T.reshape(KC, 128, S).transpose(1, 0, 2)).astype(np.float16)
    bias_groups = []
    for h in range(NH):
        bias_groups.append(bq[rows][h * DH:(h + 1) * DH])
        bias_groups.append(bk[rows][h * DH:(h + 1) * DH])
    bias_groups.append(bv[rows])
    biasrow = np.concatenate(bias_groups)[None, :].astype(np.float16)
    maskT = np.ascontiguousarray(
        attention_mask[b, 0, 0].reshape(NJ, 128).T)      # [128, NJ]
    return {"hidT": hidT, "wT": wT, "biasrow": biasrow, "maskT": maskT}


def kernel(hidden_states, attention_mask, Wq, bq, Wk, bk, Wv, bv):
    global _NC_CACHE, LAST_RESULT
    hidden_states = np.asarray(hidden_states, dtype=np.float32)
    attention_mask = np.asarray(attention_mask, dtype=np.float32)
    if _NC_CACHE is None:
        _NC_CACHE = build_nc()
    nc = _NC_CACHE
    in_maps = [
        _prep_core_inputs(c, hidden_states, attention_mask,
                          np.asarray(Wq), np.asarray(bq), np.asarray(Wk),
                          np.asarray(bk), np.asarray(Wv), np.asarray(bv))
        for c in range(N_CORES)
    ]
    res = run_bass_kernel_spmd(nc, in_maps, core_ids=list(range(N_CORES)),
                               trace=TRACE)
    LAST_RESULT = {"exec_time_ns": res.exec_time_ns,
                   "trace": res.instructions_and_trace}
    out = np.empty((B, S, H * DH), dtype=np.float32)
    for c in range(N_CORES):
        b, h0 = c // 4, NH * (c % 4)
        r = res.results[c]["out"]                        # [NH, DH, S]
        out[b, :, h0 * DH:(h0 + NH) * DH] = r.reshape(NH * DH, S).T
    return out


# revision 11
# speedup vs baseline: 2.1357x; 1.0512x over previous
"""BERT self-attention (B=2, S=2048, D=768, H=12, DH=64) on 8 trn2 NeuronCores.

Sharding: data parallel on batch x tensor parallel on heads. Core c handles
batch b = c // 4 and heads h0..h0+2 with h0 = 3 * (c % 4) — 24 (b, h) units,
3 per core.

Per-core kernel (all layouts chosen so nothing is transposed on-chip):
  - hidden^T [768, 2048] arrives k-major; W^T slices arrive as stationary
    groups. Q^T/K^T [64, 2048] come straight out of the projection matmuls
    (head dim on partitions); V comes out token-major [2048, 64] by swapping
    stationary/moving operands. Biases are folded in as one extra K=1
    accumulating matmul per output tile (bias x ones-row rank-1 update).
  - Scores are computed transposed: S^T[j, i] = K^T.T @ Q^T per 128-key block,
    so the softmax probs are already key-major for the P @ V contraction.
  - exp runs on ScalarE straight out of PSUM with the 1/sqrt(DH) scale and the
    additive attention mask fused into the activation's scale/bias. No max
    subtraction: scores here are ~N(0, 1) + mask, far from fp32 exp overflow.
  - The softmax denominator is row 64 of the P @ V matmul: V is augmented with
    a ones column, so sum_j exp(S^T[j, i]) accumulates alongside the context.
  - Normalize: reciprocal of the denominator row, broadcast across the 64 head
    dims with a K=1 matmul, multiply on VectorE while draining PSUM.
Output per core is head-major transposed [3, 64, 2048]; the host assembles the
full [B, S, D] tensor (pure unsharding/layout, no arithmetic).
"""

import numpy as np

import concourse.bass as bass
import concourse.mybir as mybir
import concourse.tile as tile
from concourse import bacc
from concourse.bass import ts, ds
from concourse.bass_utils import run_bass_kernel_spmd

B, S, D = 2, 2048, 768
H, DH = 12, 64
NH = 3            # heads per core
N_CORES = 8
KC = D // 128     # contraction chunks (6)
NJ = S // 128     # key blocks (16)
IB = 1024         # query block (i) processed per exp/PV round
MM_DT = mybir.dt.float16      # matmul operand dtype (psum accum stays f32)
TRACE = False     # set True (from test.py) to capture an NTFF profile
LAST_RESULT = {}  # exec_time_ns etc. for test.py

f32 = mybir.dt.float32
f16 = mybir.dt.float16
AF = mybir.ActivationFunctionType

_NC_CACHE = None


def build_nc():
    nc = bacc.Bacc("TRN2", target_bir_lowering=False, debug=False,
                   num_devices=N_CORES)
    hidT_d = nc.dram_tensor("hidT", [128, KC, S], MM_DT, kind="ExternalInput")
    wT_d = nc.dram_tensor("wT", [128, KC, 576], MM_DT, kind="ExternalInput")
    bias_d = nc.dram_tensor("biasrow", [1, 576], MM_DT, kind="ExternalInput")
    mask_d = nc.dram_tensor("maskT", [128, NJ], f32, kind="ExternalInput")
    out_d = nc.dram_tensor("out", [NH, DH, S], f32, kind="ExternalOutput")

    with tile.TileContext(nc) as tc:
        with (
            tc.tile_pool(name="const", bufs=1) as cpool,
            tc.tile_pool(name="proj", bufs=1) as proj,
        ):
            ones = cpool.tile([1, 512], MM_DT)
            nc.vector.memset(ones[:], 1.0)
            ones_f = cpool.tile([1, 64], f32)
            nc.vector.memset(ones_f[:], 1.0)
            biasrow = cpool.tile([1, 576], MM_DT)
            nc.sync.dma_start(biasrow[:], bias_d[:])
            maskT = cpool.tile([128, NJ], f32)
            nc.sync.dma_start(maskT[:], mask_d[:])

            # qk2 rows 0:64 = Q^T (drained), rows 64:128 = copy of Q^T; k2
            # rows 64:128 = K^T. The score matmul runs on partitions 64:128
            # (engines can't move data across partitions; psum rows 64:128
            # land there and a SB->SB DMA duplicates Q^T up).
            qk2 = proj.tile([128, NH, S], MM_DT)
            k2 = proj.tile([128, NH, S], MM_DT)
            vAug = proj.tile([128, NH, NJ, DH + 1], MM_DT)
            nc.vector.memset(vAug[:], 1.0)  # col DH stays 1.0 -> denominators

            # ---- QKV projection phase ----
            with (
                tc.tile_pool(name="hid", bufs=1) as hpool,
                tc.tile_pool(name="wts", bufs=1) as wpool,
                tc.tile_pool(name="psQK", bufs=3, space="PSUM") as psQK,
                tc.tile_pool(name="psV", bufs=3, space="PSUM") as psV,
            ):
                hidT = hpool.tile([128, KC, S], MM_DT)
                nc.sync.dma_start(hidT[:], hidT_d[:])
                wT = wpool.tile([128, KC, 576], MM_DT)
                nc.sync.dma_start(wT[:], wT_d[:])

                # Q^T/K^T: stationary = [Wq_h^T | Wk_h^T] (128 cols), moving
                # = hidden^T -> psum rows 0:64 = Q^T, 64:128 = K^T.
                for h in range(NH):
                    for t in range(S // 512):
                        ps = psQK.tile([128, 512], f32)
                        for c in range(KC):
                            nc.tensor.matmul(
                                ps[:], wT[:, c, ts(h, 128)],
                                hidT[:, c, ts(t, 512)],
                                start=(c == 0), stop=False)
                        nc.tensor.matmul(  # + [bq|bk] x ones  (K=1)
                            ps[:], biasrow[0:1, ts(h, 128)], ones[0:1, 0:512],
                            start=False, stop=True)
                        nc.vector.tensor_copy(qk2[0:64, h, ts(t, 512)],
                                              ps[0:64, :])
                        nc.vector.tensor_copy(k2[64:128, h, ts(t, 512)],
                                              ps[64:128, :])
                        nc.sync.dma_start(qk2[64:128, h, ts(t, 512)],
                                          qk2[0:64, h, ts(t, 512)])
                        nc.sync.dma_start(k2[0:64, h, ts(t, 512)],
                                          k2[64:128, h, ts(t, 512)])

                # V token-major: stationary = hidden^T chunk (tokens as cols),
                # moving = Wv^T all 3 heads (192 cols).
                for t in range(NJ):
                    ps = psV.tile([128, 192], f32)
                    for c in range(KC):
                        nc.tensor.matmul(
                            ps[:], hidT[:, c, ts(t, 128)], wT[:, c, 384:576],
                            start=(c == 0), stop=False)
                    nc.tensor.matmul(  # + ones x bv  (K=1)
                        ps[:], ones[0:1, 0:128], biasrow[0:1, 384:576],
                        start=False, stop=True)
                    nc.vector.tensor_copy(
                        vAug[:, :, t, 0:DH],
                        ps[:].rearrange("p (h d) -> p h d", h=NH))

            # ---- attention phase ----
            with (
                tc.tile_pool(name="expS", bufs=2) as epool,
                tc.tile_pool(name="psS", bufs=2, space="PSUM") as psS,
                tc.tile_pool(name="psC", bufs=2, space="PSUM") as psC,
                tc.tile_pool(name="psB", bufs=2, space="PSUM") as psB,
                tc.tile_pool(name="den", bufs=4) as dpool,
                tc.tile_pool(name="rb", bufs=3) as rpool,
                tc.tile_pool(name="ost", bufs=3) as opool,
            ):
                for h in range(NH):
                    for ib in range(S // IB):
                        eS = epool.tile([128, NJ, IB], MM_DT, tag="eS")
                        for j in range(NJ):
                            # alternate array halves so adjacent j-blocks run
                            # concurrently in the PE (row-group tiling)
                            sl = slice(0, 64) if j % 2 == 0 else slice(64, 128)
                            ps = psS.tile([128, IB], f32, tag="psS")
                            for n in range(IB // 512):
                                nc.tensor.matmul(
                                    ps[:, ts(n, 512)],
                                    k2[sl, h, ts(j, 128)],
                                    qk2[sl, h, ds(ib * IB + n * 512, 512)],
                                    start=True, stop=True)
                            # exp(S/8 + mask_j), mask as per-partition bias
                            nc.scalar.activation(
                                eS[:, j, :], ps[:], AF.Exp,
                                bias=maskT[:, j:j + 1], scale=0.125)
                        for it in range(IB // 512):
                            pc = psC.tile([DH + 1, 512], f32, tag="psC")
                            for j in range(NJ):
                                nc.tensor.matmul(
                                    pc[:], vAug[:, h, j, :],
                                    eS[:, j, ts(it, 512)],
                                    start=(j == 0), stop=(j == NJ - 1))
                            dr = dpool.tile([1, 512], f32, tag="den")
                            nc.vector.reciprocal(dr[:], pc[DH:DH + 1, :])
                            pb = psB.tile([64, 512], f32, tag="psB")
                            nc.tensor.matmul(pb[:], ones_f[0:1, 0:DH], dr[:],
                                             start=True, stop=True)
                            rB = rpool.tile([64, 512], f32, tag="rb")
                            nc.vector.tensor_copy(rB[:], pb[:])
                            o = opool.tile([64, 512], f32, tag="ost")
                            nc.vector.tensor_mul(o[:], pc[0:DH, :], rB[:])
                            nc.sync.dma_start(
                                out_d[h, :, ds(ib * IB + it * 512, 512)], o[:])
    nc.compile()
    return nc


def _prep_core_inputs(c, hidden_states, attention_mask, Wq, bq, Wk, bk, Wv, bv):
    b, h0 = c // 4, NH * (c % 4)
    rows = slice(h0 * DH, (h0 + NH) * DH)
    Wq_s, Wk_s, Wv_s = Wq[rows], Wk[rows], Wv[rows]      # [192, 768] each
    groups = []
    for h in range(NH):
        groups.append(Wq_s[h * DH:(h + 1) * DH])
        groups.append(Wk_s[h * DH:(h + 1) * DH])
    groups.append(Wv_s)
    big = np.concatenate(groups, axis=0)                 # [576, 768]
    wT = np.ascontiguousarray(
        big.T.reshape(KC, 128, 576).transpose(1, 0, 2)).astype(np.float16)
    hidT = np.ascontiguousarray(
        hidden_states[b].T.reshape(KC, 128, S).transpose(1, 0, 2)).astype(np.float16)
    bias_groups = []
    for h in range(NH):
        bias_groups.append(bq[rows][h * DH:(h + 1) * DH])
        bias_groups.append(bk[rows][h * DH:(h + 1) * DH])
    bias_groups.append(bv[rows])
    biasrow = np.concatenate(bias_groups)[None, :].astype(np.float16)
    maskT = np.ascontiguousarray(
        attention_mask[b, 0, 0].reshape(NJ, 128).T)      # [128, NJ]
    return {"hidT": hidT, "wT": wT, "biasrow": biasrow, "maskT": maskT}


def kernel(hidden_states, attention_mask, Wq, bq, Wk, bk, Wv, bv):
    global _NC_CACHE, LAST_RESULT
    hidden_states = np.asarray(hidden_states, dtype=np.float32)
    attention_mask = np.asarray(attention_mask, dtype=np.float32)
    if _NC_CACHE is None:
        _NC_CACHE = build_nc()
    nc = _NC_CACHE
    in_maps = [
        _prep_core_inputs(c, hidden_states, attention_mask,
                          np.asarray(Wq), np.asarray(bq), np.asarray(Wk),
                          np.asarray(bk), np.asarray(Wv), np.asarray(bv))
        for c in range(N_CORES)
    ]
    res = run_bass_kernel_spmd(nc, in_maps, core_ids=list(range(N_CORES)),
                               trace=TRACE)
    LAST_RESULT = {"exec_time_ns": res.exec_time_ns,
                   "trace": res.instructions_and_trace}
    out = np.empty((B, S, H * DH), dtype=np.float32)
    for c in range(N_CORES):
        b, h0 = c // 4, NH * (c % 4)
        r = res.results[c]["out"]                        # [NH, DH, S]
        out[b, :, h0 * DH:(h0 + NH) * DH] = r.reshape(NH * DH, S).T
    return out
